# revision 44
# baseline (speedup 1.0000x reference)
"""MinLSTM layer on 8 Trainium2 NeuronCores.

Math (equivalent to the log-space reference, done in linear space):
    f_pre = x @ W_f.T + b_f ; i_pre = x @ W_i.T + b_i ; h_pre = x @ W_h.T + b_h
    sf = sigmoid(f_pre) ; si = sigmoid(i_pre)
    f = sf / (sf + si)                       # normalized forget gate
    i = 1 - f                                # = si / (sf + si)
    g = max(sigmoid(h_pre), h_pre + 0.5)     # == exp(log_g), exactly
    h_t = f_t * h_{t-1} + i_t * g_t,  h_0 = 1
The gates satisfy f in (0,1), g > 0, so h stays in a tame range and the
recurrence is numerically stable in fp32 (max rel err vs the fp32 log-space
reference ~1e-3 with fp16 matmul operands; fp32 PSUM accumulation).

Sharding: 8 cores = batch(4) x hidden-halves(2). Core c handles batch b=c//2,
hidden slice [(c%2)*512, (c%2+1)*512). No cross-core communication; the scan
runs along T inside each core via the DVE TensorTensorScan instruction
(state = f*state - mv per step, mv = (f-1)*g = -i*g).

Device layout: gates computed as [h_part, t_free] via out = W_sliceT.T @ xT;
host pre-transposes/packs x and W (numpy) and re-transposes the [512, 4096]
per-core output back to [T, Dh].

Schedule (trace-driven; see kernel_baseline.py for the previous version):
- The matmul stream floor is 768 x 216ns = 166us; everything else must hide
  under it. Trace facts this schedule is built on: a dma_start occupies its
  queue ~600ns regardless of size and the first HBM transfer rides a ~5us
  ring-start/ramp; the PE runs at half clock for ~3us after any idle hole
  in its instruction stream; DVE op costs at 512 cols are ~690-750ns
  (add/mul/stt) and ~1280ns (scan), GPSIMD ~1260ns per op.
- The first (x0, W_f) k-pair rides TWO cold DMA rings in parallel (x00 on
  Sync, Wf0 on the otherwise-idle Scalar ring): each ring pays its ~4.5us
  ring-start concurrently, so the pair lands ~11.8us instead of ~13us.
  Both tiles also read from dedicated sequential-layout copies (x00c/wf0c
  extra inputs) rather than the 64KB/8KB-strided main pack — sequential
  HBM reads deliver sooner on the cold ring.
  Everything else rides the single Sync queue (sustained parallel queues
  deliver LESS aggregate than one), ordered (x0 W_f W_i) per k-pair, then
  W_h, then x chunk 1, pacing the J0 consumption. Bias rides the idle
  GPSIMD queue. x chunks 0,1 are four 256KB tiles each (fine-grained early
  dependencies); chunks 2..7 are one 1MB DMA each, prefetched two ahead.
- Warmup matmuls (9 x 512-col + 4 x 256-col for a finer-grained boundary)
  on a GPSIMD-memset scratch tile keep the PE stream busy (and its clock
  ramping) from queue-open (~7.5us) until the first x/W pair lands; the
  real stream then runs gap-free at 216ns per matmul to the end.
- J0 (chunk 0) runs the f and i gates k-interleaved (their 8 PSUM banks
  drain in sigmoid(f0) sigmoid(f1) order so the h gate can reuse banks
  immediately), then the h gate k-outer while W_h arrives.
- Steady chunks 1..6 are unit-major (f/i/h matmuls back-to-back per
  h-tile); the f/i normalization add+mul runs on the otherwise-idle GPSIMD
  so the DVE (reciprocal + g/mv/scan, ~3.4us per unit vs 5.2us of PE) has
  slack and enters the final chunk without backlog.
- Final chunk interleaves f/i and h groups (fi0 fi1 h0 fi2 h1 h2 fi3 h3);
  unit 3's normalize mul runs on GPSIMD so it overlaps gt on the DVE, and
  after the last matmul only sigmoid(h) -> g -> mv -> scan -> store for
  unit 3 remains (~4.8us vs ~6.2us before).
- The scan writes fp16 output tiles (the DVE computes the recurrence in
  fp32 internally; only the stored values and the inter-chunk carry are
  rounded), halving output HBM traffic; the host upcasts to fp32. Max rel
  err 1.34e-3 vs the log-space fp32 reference (gate: 2e-2).
"""

import sys

for _p in ("/opt/trn_rl_repo",):
    if _p not in sys.path:
        sys.path.append(_p)

import numpy as np

import concourse.bass as bass
import concourse.tile as tile
from concourse import bacc, mybir
from concourse.bass_utils import run_bass_kernel_spmd

B, T, DIN, DH = 4, 4096, 1024, 1024
N_CORES = 8
HSH = DH // 2          # 512 hidden channels per core
P = 128                # partitions
KT = DIN // P          # 8 contraction tiles
KK = KT // 2           # 4 packed k-pairs (2KB DMA lines)
NT = 512               # matmul t-chunk (free dim, one PSUM bank)
NC = T // NT           # 8 t-chunks
IT = HSH // P          # 4 h-tiles per core
N_WARM = 9             # warmup matmuls (PE clock ramp + pre-data fill)

MM_DT = mybir.dt.float16
MM_NP = np.float16

_COMPILED = None


def _build():
    AF = mybir.ActivationFunctionType
    OP = mybir.AluOpType
    f32 = mybir.dt.float32

    nc = bacc.Bacc("TRN2", target_bir_lowering=False, debug=False)

    # x packed as [p, c, kk, (j t)]: j = k-pair half, t in 0..511
    xT = nc.dram_tensor("xT", [P, T * KT], MM_DT, kind="ExternalInput").ap()
    x_f = xT.rearrange("p (c kk tt) -> p c kk tt", c=NC, kk=KK)   # fine view
    x_b = xT.rearrange("p (c r) -> p c r", c=NC)                  # big view
    # W packed as [p, kk, (j h)]: 2KB per partition line
    wd = {g: nc.dram_tensor(f"w{g}", [P, KK * 2 * HSH], MM_DT,
                            kind="ExternalInput").ap()
          for g in ("f", "i", "h")}
    w_v = {g: w.rearrange("p (kk z) -> p kk z", kk=KK) for g, w in wd.items()}
    # contiguous copies of the first x / W_f k-pair tiles: the regular pack
    # is 2KB lines at 64KB/8KB stride, and the first transfers ride the cold
    # DMA-ring/HBM ramp where sequential reads deliver sooner. These two
    # tiles bound the stream start, so they get a sequential-layout copy.
    x00c = nc.dram_tensor("x00c", [P, 2 * NT], MM_DT,
                          kind="ExternalInput").ap()
    wf0c = nc.dram_tensor("wf0c", [P, 2 * HSH], MM_DT,
                          kind="ExternalInput").ap()
    # packed per-partition scalars: [b_f | b_i | b_h | b_h+0.5], each (128, IT)
    biases = nc.dram_tensor("biases", [P, 4 * IT], f32, kind="ExternalInput").ap()
    out = nc.dram_tensor("out", [HSH, T], mybir.dt.float16,
                         kind="ExternalOutput").ap()

    with tile.TileContext(nc) as tc:
        with (
            tc.tile_pool(name="wpool", bufs=1) as wpool,
            tc.tile_pool(name="bpool", bufs=1) as bpool,
            tc.tile_pool(name="xfpool", bufs=8) as xfpool,
            tc.tile_pool(name="xbpool", bufs=4) as xbpool,
            tc.tile_pool(name="psum", bufs=8, space="PSUM") as pspool,
            tc.tile_pool(name="work", bufs=10) as work,
            tc.tile_pool(name="ework", bufs=4) as ework,
            tc.tile_pool(name="hpool", bufs=6) as hpool,
        ):
            bias_t = bpool.tile([P, 4 * IT], f32, tag="bias")

            # per-kk weight tiles ([128, 1024] = both k halves), resident
            wt = {g: [wpool.tile([P, 2 * HSH], MM_DT, tag=f"w{g}{kk}",
                                 name=f"w{g}{kk}_t")
                      for kk in range(KK)] for g in ("f", "i", "h")}

            def wsl(g, k, i):
                kk, j = divmod(k, 2)
                c0 = j * HSH + i * P
                return wt[g][kk][:, c0:c0 + P]

            def bias_ap(kind, i):
                return bias_t[:, kind * IT + i:kind * IT + i + 1]

            # ---- warmups: scratch memset on the (idle) GPSIMD engine so
            # the PE queue can start ramping as early as possible ----
            scratch = bpool.tile([P, NT], MM_DT, tag="scratch")
            nc.gpsimd.memset(scratch[:].bitcast(mybir.dt.uint32), 0)
            pswarm = pspool.tile([P, NT], f32, tag="ps", name="pswarm_t")
            for _ in range(N_WARM):
                nc.tensor.matmul(pswarm[:], lhsT=scratch[:, :P], rhs=scratch[:],
                                 start=True, stop=True)
            for _ in range(4):      # finer-grained warmup tail (256 cols)
                nc.tensor.matmul(pswarm[:, :256], lhsT=scratch[:, :P],
                                 rhs=scratch[:, :256], start=True, stop=True)

            # ---- DMA issue: everything on the single Sync queue, ordered
            # so the delivery rate (~0.65us/256KB tile after the HBM ramp)
            # matches the PE's consumption: (x0 W_f W_i) per k-pair — J0
            # consumes one such triple per ~1.7us of f/i matmuls — then
            # W_h, then x chunk 1. Bias rides the idle GPSIMD queue. ----
            xfine = {0: [], 1: []}
            for c in (0, 1):
                for kk in range(KK):
                    xfine[c].append(xfpool.tile([P, 2 * NT], MM_DT, tag="xf",
                                                name="xf_t"))
            nc.sync.dma_start(out=xfine[0][0][:], in_=x00c)
            nc.scalar.dma_start(out=wt["f"][0][:], in_=wf0c)
            nc.gpsimd.dma_start(out=bias_t[:], in_=biases[:])
            nc.sync.dma_start(out=wt["i"][0][:], in_=w_v["i"][:, 0, :])
            for kk in range(1, KK):
                nc.sync.dma_start(out=xfine[0][kk][:], in_=x_f[:, 0, kk, :])
                nc.sync.dma_start(out=wt["f"][kk][:], in_=w_v["f"][:, kk, :])
                nc.sync.dma_start(out=wt["i"][kk][:], in_=w_v["i"][:, kk, :])
            for kk in range(KK):
                nc.sync.dma_start(out=wt["h"][kk][:], in_=w_v["h"][:, kk, :])
            for kk in range(KK):
                nc.sync.dma_start(out=xfine[1][kk][:], in_=x_f[:, 1, kk, :])

            xbig = {}

            def fetch_big(c):
                if c < NC and c not in xbig:
                    xt = xbpool.tile([P, KT * NT], MM_DT, tag="xb", name="xb_t")
                    nc.sync.dma_start(out=xt[:], in_=x_b[:, c, :])
                    xbig[c] = xt

            fetch_big(2)
            fetch_big(3)

            def xsl(c, k, sl=slice(0, NT)):
                kk, j = divmod(k, 2)
                if c in xfine:
                    return xfine[c][kk][:, j * NT + sl.start:j * NT + sl.stop]
                o = kk * 2 * NT + j * NT
                return xbig[c][:, o + sl.start:o + sl.stop]

            def mm_gate(g, c, u, sl=slice(0, NT)):
                """8 k-matmuls of gate g, h-tile u, chunk c into a psum tile."""
                n = sl.stop - sl.start
                pst = pspool.tile([P, n], f32, tag="ps", name="ps_t")
                for k in range(KT):
                    nc.tensor.matmul(pst[:], lhsT=wsl(g, k, u),
                                     rhs=xsl(c, k, sl),
                                     start=(k == 0), stop=(k == KT - 1))
                return pst

            def act_sig(dst, pst, bk, u):
                nc.scalar.activation(dst[:], pst[:], AF.Sigmoid,
                                     bias=bias_ap(bk, u), scale=1.0)

            def normalize(sf, si, tt, n=NT):
                """f = sf/(sf+si); add+mul on engine tt, recip on DVE."""
                tt.tensor_add(si[:], sf[:], si[:])
                r = ework.tile([P, n], f32, tag="r", name="r_t")
                nc.vector.reciprocal_approx_fast(out=r[:], in_=si[:])
                tt.tensor_mul(sf[:], sf[:], r[:])

            def h_drain(c, u, pst, sf, t0, sl=slice(0, NT)):
                """sigmoid(h) -> g -> mv -> scan -> store for one unit.

                sf is the (already normalized) forget gate tile; only its
                [sl] columns are consumed, matching pst's width."""
                n = sl.stop - sl.start
                sfv = sf[:, sl.start:sl.stop]
                sg = ework.tile([P, n], f32, tag="sg", name="sg_t")
                nc.scalar.activation(sg[:], pst[:], AF.Sigmoid,
                                     bias=bias_ap(2, u), scale=1.0)
                gt = ework.tile([P, n], f32, tag="gt", name="gt_t")
                nc.vector.scalar_tensor_tensor(           # g = max(pre+bh5, sg)
                    gt[:], pst[:], bias_ap(3, u), sg[:],
                    op0=OP.add, op1=OP.max)
                nc.vector.scalar_tensor_tensor(           # mv = (f-1)*g
                    gt[:], sfv, 1.0, gt[:],
                    op0=OP.subtract, op1=OP.mult)
                hc = hpool.tile([P, n], MM_DT, tag="h", name=f"h{u}_t")
                init = 1.0 if (c == 0 and sl.start == 0) else hprev[u][:, -1:]
                nc.vector.tensor_tensor_scan(
                    hc[:], sfv, gt[:], init, op0=OP.mult, op1=OP.subtract)
                hprev[u] = hc
                nc.sync.dma_start(
                    out=out[u * P:(u + 1) * P,
                            t0 + sl.start:t0 + sl.stop],
                    in_=hc[:])

            hprev = [None] * IT

            # ---- J0 (chunk 0): f and i gates k-interleaved (8 PSUM banks)
            # so the PE consumes one (x, W_f, W_i) tile-triple per ~1.7us,
            # matching the HBM ramp delivery rate; then the h gate k-outer
            # while W_h arrives ----
            psf = [pspool.tile([P, NT], f32, tag="ps", name="ps_t")
                   for _ in range(IT)]
            psi = [pspool.tile([P, NT], f32, tag="ps", name="ps_t")
                   for _ in range(IT)]
            for k in range(KT):
                for u in range(IT):
                    nc.tensor.matmul(psf[u][:], lhsT=wsl("f", k, u),
                                     rhs=xsl(0, k),
                                     start=(k == 0), stop=(k == KT - 1))
                for u in range(IT):
                    nc.tensor.matmul(psi[u][:], lhsT=wsl("i", k, u),
                                     rhs=xsl(0, k),
                                     start=(k == 0), stop=(k == KT - 1))
            # ACT drain order frees the PSUM banks the h-phase reuses just
            # in time: sigmoid(f0), sigmoid(f1) first so h(u0,u1) can start
            # while the remaining f/i activations drain.
            sfA = [work.tile([P, NT], f32, tag="sf", name="sf_t")
                   for _ in range(IT)]
            siA = [work.tile([P, NT], f32, tag="si", name="si_t")
                   for _ in range(IT)]
            act_sig(sfA[0], psf[0], 0, 0)
            act_sig(sfA[1], psf[1], 0, 1)
            act_sig(siA[0], psi[0], 1, 0)
            act_sig(siA[1], psi[1], 1, 1)
            psh = [pspool.tile([P, NT], f32, tag="ps", name="ps_t")
                   for _ in range(2)]
            for k in range(KT):
                for u in range(2):
                    nc.tensor.matmul(psh[u][:], lhsT=wsl("h", k, u),
                                     rhs=xsl(0, k),
                                     start=(k == 0), stop=(k == KT - 1))
            act_sig(sfA[2], psf[2], 0, 2)
            act_sig(sfA[3], psf[3], 0, 3)
            act_sig(siA[2], psi[2], 1, 2)
            act_sig(siA[3], psi[3], 1, 3)
            for u in range(IT):
                normalize(sfA[u], siA[u], nc.vector)
            psh += [pspool.tile([P, NT], f32, tag="ps", name="ps_t")
                    for _ in range(2)]
            for k in range(KT):
                for u in (2, 3):
                    nc.tensor.matmul(psh[u][:], lhsT=wsl("h", k, u),
                                     rhs=xsl(0, k),
                                     start=(k == 0), stop=(k == KT - 1))
            for u in range(IT):
                h_drain(0, u, psh[u], sfA[u], 0)

            # ---- steady phase: chunks 1..6, unit-major ----
            for c in range(1, NC - 1):
                fetch_big(c + 2)
                for u in range(IT):
                    pf = mm_gate("f", c, u)
                    sf = work.tile([P, NT], f32, tag="sf", name="sf_t")
                    act_sig(sf, pf, 0, u)
                    pi = mm_gate("i", c, u)
                    si = work.tile([P, NT], f32, tag="si", name="si_t")
                    act_sig(si, pi, 1, u)
                    ph = mm_gate("h", c, u)
                    normalize(sf, si, nc.gpsimd)
                    h_drain(c, u, ph, sf, c * NT)

            # ---- final chunk: interleave f/i groups with the previous
            # units' h groups (fi0 fi1 h0 fi2 h1 fi3 h2 h3) so each unit's
            # normalization and drain hides under later units' matmuls and
            # only one sigmoid -> g -> mv -> scan -> store chain follows
            # the last matmul ----
            c = NC - 1

            def fi_unit(u):
                pf = mm_gate("f", c, u)
                sf = work.tile([P, NT], f32, tag="sf", name="sf_t")
                act_sig(sf, pf, 0, u)
                pi = mm_gate("i", c, u)
                si = work.tile([P, NT], f32, tag="si", name="si_t")
                act_sig(si, pi, 1, u)
                if u == 3:
                    # unit 3's normalize tail runs concurrently with the
                    # last h matmuls: keep add+recip on the DVE but put the
                    # mul on GPSIMD so it overlaps gt3 on the DVE
                    nc.vector.tensor_add(si[:], sf[:], si[:])
                    r = ework.tile([P, NT], f32, tag="r", name="r_t")
                    nc.vector.reciprocal_approx_fast(out=r[:], in_=si[:])
                    nc.gpsimd.tensor_mul(sf[:], sf[:], r[:])
                else:
                    normalize(sf, si, nc.vector)
                return sf

            def h_unit(u, sf):
                ph = mm_gate("h", c, u)
                h_drain(c, u, ph, sf, c * NT)

            sf0 = fi_unit(0)
            sf1 = fi_unit(1)
            h_unit(0, sf0)
            sf2 = fi_unit(2)
            h_unit(1, sf1)
            h_unit(2, sf2)
            sf3 = fi_unit(3)
            h_unit(3, sf3)

    nc.compile()
    return nc


def _in_maps(x, W_f, b_f, W_i, b_i, W_h, b_h):
    x = np.asarray(x, MM_NP)
    wT = {g: np.asarray(w, np.float32).T.astype(MM_NP)
          for g, w in (("f", W_f), ("i", W_i), ("h", W_h))}
    bs = {g: np.asarray(b, np.float32) for g, b in (("f", b_f), ("i", b_i), ("h", b_h))}

    maps = []
    for c in range(N_CORES):
        b, hh = divmod(c, 2)
        hsl = slice(hh * HSH, (hh + 1) * HSH)
        bias_pack = np.concatenate([
            bs["f"][hsl].reshape(IT, P).T,
            bs["i"][hsl].reshape(IT, P).T,
            bs["h"][hsl].reshape(IT, P).T,
            (bs["h"][hsl] + 0.5).reshape(IT, P).T,
        ], axis=1)
        # x pack: [p, c, kk, j, t] = xT[(2kk+j)*P + p, c*NT + t]
        xb = np.ascontiguousarray(x[b].T)                    # (DIN, T)
        xp = xb.reshape(KK, 2, P, NC, NT).transpose(2, 3, 0, 1, 4)
        # W pack: [p, kk, j, h] = W^T[(2kk+j)*P + p, h]
        wp = {g: wT[g][:, hsl].reshape(KK, 2, P, HSH).transpose(2, 0, 1, 3)
              for g in ("f", "i", "h")}
        xp_flat = np.ascontiguousarray(xp.reshape(P, T * KT))
        wf_flat = np.ascontiguousarray(wp["f"].reshape(P, KK * 2 * HSH))
        maps.append({
            "xT": xp_flat,
            "x00c": np.ascontiguousarray(xp_flat[:, :2 * NT]),
            "wf0c": np.ascontiguousarray(wf_flat[:, :2 * HSH]),
            "wf": wf_flat,
            "wi": np.ascontiguousarray(wp["i"].reshape(P, KK * 2 * HSH)),
            "wh": np.ascontiguousarray(wp["h"].reshape(P, KK * 2 * HSH)),
            "biases": np.ascontiguousarray(bias_pack, dtype=np.float32),
        })
    return maps


def kernel(x, W_f, b_f, W_i, b_i, W_h, b_h):
    global _COMPILED
    if _COMPILED is None:
        _COMPILED = _build()
    nc = _COMPILED

    res = run_bass_kernel_spmd(
        nc, _in_maps(x, W_f, b_f, W_i, b_i, W_h, b_h), list(range(N_CORES)))

    full = np.empty((B, T, DH), np.float32)
    for c in range(N_CORES):
        b, hh = divmod(c, 2)
        full[b, :, hh * HSH:(hh + 1) * HSH] = res.results[c]["out"].T
    return full
